# revision 3
# baseline (speedup 1.0000x reference)
"""FPN + RPN detector kernel for Trainium2, 8-core SPMD.

Sharding: core = (image b in {0,1}) x (row-quarter q in {0..3}). Host slices
inputs with halos (zero-padded), every core runs the same program on its
slice, host crops/reassembles. No cross-core communication.

All convs are float32r matmuls (full PE rate at N>=256), channels on
partitions in two halves of 128, PSUM f32 accumulation over taps/halves.
"""
import numpy as np

N_CORES = 8

# Quarter starts chosen so every upsample/subsample alignment offset is
# uniform across cores (SPMD requirement).
S2 = [0, 48, 96, 144]
S3 = [0, 24, 48, 72]
S4 = [0, 12, 24, 36]
S5 = [0, 6, 12, 18]
S6 = [0, 3, 6, 9]
H = [200, 100, 50, 25, 13]
W = [200, 100, 50, 25, 13]
OWN = [56, 28, 14, 7, 4]
VALID = [
    [48, 48, 48, 56],
    [24, 24, 24, 28],
    [12, 12, 12, 14],
    [6, 6, 6, 7],
    [3, 3, 3, 4],
]
M_ROWS = [60, 32, 18, 13]     # m2..m5 region rows (m_l from S_l-2, m5 S5-3)
CIN = [256, 512, 1024, 2048]
KH = [2, 4, 8, 16]
CW = [200, 100, 50, 26]       # c-slice widths (c5 x-padded to 26)
CONVW = [200, 100, 50, 26, 14]  # conv window x-counts (even)

_CACHED = {}


def _build():
    import concourse.bacc as bacc
    import concourse.mybir as mybir
    from concourse.tile import TileContext

    F32, F32R = mybir.dt.float32, mybir.dt.float32r
    AF = mybir.ActivationFunctionType
    ALU = mybir.AluOpType

    nc = bacc.Bacc("TRN2", target_bir_lowering=False, debug=False,
                   num_devices=N_CORES)

    c_d = [nc.dram_tensor(f"c{i}", [KH[i], 128, M_ROWS[i], CW[i]], F32R,
                          kind="ExternalInput") for i in range(4)]
    wlat_d = [nc.dram_tensor(f"wlat{i}", [KH[i], 128, 2, 128], F32R,
                             kind="ExternalInput") for i in range(4)]
    wtr_d = [nc.dram_tensor(f"wtr{i}", [2, 128, 2, 3, 3, 128], F32R,
                            kind="ExternalInput") for i in range(4)]
    wrpn1_d = nc.dram_tensor("wrpn1", [2, 128, 2, 3, 3, 128], F32R,
                             kind="ExternalInput")
    wrpn2_d = nc.dram_tensor("wrpn2", [2, 128, 16], F32R, kind="ExternalInput")
    btr_d = nc.dram_tensor("btr", [4, 128, 2], F32, kind="ExternalInput")
    brpn1_d = nc.dram_tensor("brpn1", [128, 2], F32, kind="ExternalInput")
    brpn2_d = nc.dram_tensor("brpn2", [128, 16], F32, kind="ExternalInput")
    zeros_d = nc.dram_tensor("zeros", [128, 512], F32R, kind="ExternalInput")
    masks_d = nc.dram_tensor("masks", [128, 244], F32, kind="ExternalInput")
    MOFF = {}
    _off = 0
    for name, n in [("m2", 60), ("p2", 58), ("m3", 32), ("p3", 30),
                    ("m4", 18), ("p4", 16), ("m5", 13), ("p5", 11),
                    ("p6", 6)]:
        MOFF[name] = _off
        _off += n

    o_d = [nc.dram_tensor(f"o{i}", [OWN[i] * W[i] * 3 * 5], F32,
                          kind="ExternalOutput") for i in range(5)]

    with TileContext(nc) as tc:
        with (
            tc.tile_pool(name="const", bufs=1) as cpool,
            tc.tile_pool(name="psum", bufs=4, space="PSUM") as pspool,
            tc.tile_pool(name="pso", bufs=2, space="PSUM") as psopool,
        ):
            tz = cpool.tile([128, 512], F32R, tag="zeros")
            tmask = cpool.tile([128, 244], F32, tag="masks")
            twr1 = cpool.tile([128, 2, 2, 3, 3, 128], F32R, tag="wrpn1")
            twr2 = cpool.tile([128, 2, 16], F32R, tag="wrpn2")
            tbtr = cpool.tile([128, 4, 2], F32, tag="btr")
            tbr1 = cpool.tile([128, 2], F32, tag="brpn1")
            tbr2 = cpool.tile([128, 16], F32, tag="brpn2")
            nc.sync.dma_start(out=tz[:], in_=zeros_d[:])
            nc.sync.dma_start(out=tmask[:], in_=masks_d[:])
            for ih in range(2):
                nc.sync.dma_start(out=twr1[:, ih], in_=wrpn1_d[ih])
                nc.sync.dma_start(out=twr2[:, ih], in_=wrpn2_d[ih])
            nc.sync.dma_start(out=tbtr[:], in_=btr_d.rearrange("l p t -> p l t"))
            nc.sync.dma_start(out=tbr1[:], in_=brpn1_d[:])
            nc.sync.dma_start(out=tbr2[:], in_=brpn2_d[:])

            tm3 = cpool.tile([128, 2, 32, 102], F32R, tag="m3")
            tm4 = cpool.tile([128, 2, 18, 52], F32R, tag="m4")
            tm5 = cpool.tile([128, 2, 13, 28], F32R, tag="m5")
            tstage = cpool.tile([128, 24, 16], F32, tag="ostage")

            def zero_cols(t, nrows, cols):
                for ih in range(2):
                    for c0 in cols:
                        nc.vector.tensor_copy(
                            t[:, ih, :, c0].squeeze(), tz[:, :nrows])

            def mask_rows(t, moff, rows, i0, i1):
                for ih in range(2):
                    for r in rows:
                        mk = tmask[:, moff + r:moff + r + 1].broadcast_to(
                            [128, i1 - i0])
                        nc.vector.tensor_tensor(
                            t[:, ih, r, i0:i1], t[:, ih, r, i0:i1], mk,
                            ALU.mult)

            def conv3x3(dst_tile, groups, src_tile, src_row_of, lhsT,
                        bias, relu, lvl, mask_edge=None):
                """3x3 conv: dst row j reads src rows j+src_row_of+dy.

                relu=True -> unpadded h tile, else padded p tile (interior
                cols 1..W). groups = [(j0, nrows)], nrows*CONVW[lvl] <= 512.
                """
                wv, wl = CONVW[lvl], W[lvl]
                for (j0, nr) in groups:
                    for mo in range(2):
                        ps = pspool.tile([128, 512], F32, tag="ps")
                        n = nr * wv
                        first = True
                        for ih in range(2):
                            for dy in range(3):
                                for dx in range(3):
                                    rhs = src_tile[
                                        :, ih,
                                        j0 + src_row_of + dy:
                                        j0 + src_row_of + dy + nr,
                                        dx:dx + wv]
                                    nc.tensor.matmul(
                                        ps[:, :n], lhsT[:, ih, mo, dy, dx],
                                        rhs, start=first,
                                        stop=(ih == 1 and dy == 2 and dx == 2))
                                    first = False
                        psv = ps[:, :n].rearrange("p (r x) -> p r x", x=wv)
                        if relu:
                            dst = dst_tile[:, mo, j0:j0 + nr, :wl]
                            nc.scalar.activation(dst, psv[:, :, :wl], AF.Relu,
                                                 bias=bias[:, mo:mo + 1])
                        else:
                            dst = dst_tile[:, mo, j0:j0 + nr, 1:1 + wl]
                            nc.scalar.activation(dst, psv[:, :, :wl],
                                                 AF.Identity,
                                                 bias=bias[:, mo:mo + 1])
                if mask_edge is not None:
                    nrt, moff = mask_edge
                    mask_rows(dst_tile, moff, [0, 1, nrt - 2, nrt - 1],
                              1, 1 + wl)

            def out_head(h_tile, npix, lvl, px0):
                hflat = h_tile.rearrange("p t r x -> p t (r x)")
                for g in range((npix + 127) // 128):
                    p0 = g * 128
                    mpix = min(128, npix - p0)
                    pso = psopool.tile([128, 16], F32, tag="pso")
                    for ih in range(2):
                        nc.tensor.matmul(
                            pso[:mpix], hflat[:, ih, p0:p0 + mpix],
                            twr2[:, ih], start=(ih == 0), stop=(ih == 1))
                    st = tstage[:mpix, g % 24]
                    nc.vector.tensor_tensor(st, pso[:mpix], tbr2[:mpix],
                                            ALU.add)
                    px = px0 + p0
                    dst = o_d[lvl][px * 15:(px + mpix) * 15].rearrange(
                        "(pix c) -> pix c", c=15)
                    nc.sync.dma_start(out=dst, in_=st[:, :15])

            # ================= L5 (+L6) =================
            with tc.tile_pool(name="l5", bufs=1) as pool:
                tc5 = pool.tile([128, 16, 13, 26], F32R, tag="c5")
                twl5 = pool.tile([128, 16, 2, 128], F32R, tag="wl5")
                twt5 = pool.tile([128, 2, 2, 3, 3, 128], F32R, tag="wt5")
                tp5 = pool.tile([128, 2, 11, 28], F32R, tag="p5")
                tp6 = pool.tile([128, 2, 6, 16], F32R, tag="p6")
                th5 = pool.tile([128, 2, 7, 25], F32R, tag="h5")
                th6 = pool.tile([128, 2, 4, 13], F32R, tag="h6")
                for kh in range(16):
                    nc.sync.dma_start(out=tc5[:, kh], in_=c_d[3][kh])
                    nc.sync.dma_start(out=twl5[:, kh], in_=wlat_d[3][kh])
                for ih in range(2):
                    nc.sync.dma_start(out=twt5[:, ih], in_=wtr_d[3][ih])
                zero_cols(tm5, 13, [0, 26, 27])
                zero_cols(tp5, 11, [0, 26, 27])
                zero_cols(tp6, 6, [0, 14, 15])

                for mo in range(2):
                    ps = pspool.tile([128, 512], F32, tag="ps")
                    n = 13 * 26
                    for kh in range(16):
                        nc.tensor.matmul(
                            ps[:, :n], twl5[:, kh, mo],
                            tc5[:, kh].rearrange("p r x -> p (r x)"),
                            start=(kh == 0), stop=(kh == 15))
                    psv = ps[:, :n].rearrange("p (r x) -> p r x", x=26)
                    nc.scalar.activation(tm5[:, mo, :, 1:26], psv[:, :, :25],
                                         AF.Copy, bias=0.0)
                mask_rows(tm5, MOFF["m5"], [0, 1, 2, 10, 11, 12], 1, 26)

                conv3x3(tp5, [(0, 11)], tm5, 0, twt5, tbtr[:, 3], False, 3,
                        mask_edge=(11, MOFF["p5"]))
                # p6 = p5[::2, ::2]: row j <- p5 row 2j, col x <- p5 col 1+2x
                for ih in range(2):
                    for j in range(6):
                        src = tp5[:, ih, 2 * j, 1:27].rearrange(
                            "p (x s) -> p x s", s=2)[:, :, 0].squeeze()
                        nc.vector.tensor_copy(tp6[:, ih, j, 1:14], src)
                conv3x3(th5, [(0, 7)], tp5, 1, twr1, tbr1, True, 3)
                conv3x3(th6, [(0, 4)], tp6, 0, twr1, tbr1, True, 4)
                out_head(th5, 7 * 25, 3, 0)
                out_head(th6, 4 * 13, 4, 0)

            # ================= L4 =================
            with tc.tile_pool(name="l4", bufs=1) as pool:
                tc4 = pool.tile([128, 8, 18, 50], F32R, tag="c4")
                twl4 = pool.tile([128, 8, 2, 128], F32R, tag="wl4")
                twt4 = pool.tile([128, 2, 2, 3, 3, 128], F32R, tag="wt4")
                tp4 = pool.tile([128, 2, 16, 52], F32R, tag="p4")
                th4 = pool.tile([128, 2, 14, 50], F32R, tag="h4")
                for kh in range(8):
                    nc.sync.dma_start(out=tc4[:, kh], in_=c_d[2][kh])
                    nc.sync.dma_start(out=twl4[:, kh], in_=wlat_d[2][kh])
                for ih in range(2):
                    nc.sync.dma_start(out=twt4[:, ih], in_=wtr_d[2][ih])
                zero_cols(tm4, 18, [0, 51])
                zero_cols(tp4, 16, [0, 51])

                for (r0, nr) in [(0, 10), (10, 8)]:
                    for mo in range(2):
                        ps = pspool.tile([128, 512], F32, tag="ps")
                        n = nr * 50
                        for kh in range(8):
                            rhs = tc4[:, kh, r0:r0 + nr, :].rearrange(
                                "p r x -> p (r x)")
                            nc.tensor.matmul(ps[:, :n], twl4[:, kh, mo], rhs,
                                             start=(kh == 0), stop=(kh == 7))
                        ps5 = ps[:, :n].rearrange(
                            "p (hh r wh s) -> p r hh wh s", r=2, wh=25, s=2)
                        dest5 = tm4[:, mo, r0:r0 + nr, 1:51].rearrange(
                            "p (hh r) (wh s) -> p r hh wh s", r=2, s=2)
                        srow = 2 + r0 // 2
                        srcb = tm5[:, mo, srow:srow + nr // 2, 1:26
                                   ].unsqueeze(3).broadcast_to(
                            [128, nr // 2, 25, 2])
                        for par in range(2):
                            nc.vector.tensor_tensor(
                                dest5[:, par], ps5[:, par], srcb, ALU.add)
                mask_rows(tm4, MOFF["m4"], [0, 1, 16, 17], 1, 51)
                conv3x3(tp4, [(0, 8), (8, 8)], tm4, 0, twt4, tbtr[:, 2],
                        False, 2, mask_edge=(16, MOFF["p4"]))
                conv3x3(th4, [(0, 8), (8, 6)], tp4, 0, twr1, tbr1, True, 2)
                out_head(th4, 14 * 50, 2, 0)

            # ================= L3 =================
            with tc.tile_pool(name="l3", bufs=1) as pool:
                tc3 = pool.tile([128, 4, 32, 100], F32R, tag="c3")
                twl3 = pool.tile([128, 4, 2, 128], F32R, tag="wl3")
                twt3 = pool.tile([128, 2, 2, 3, 3, 128], F32R, tag="wt3")
                tp3 = pool.tile([128, 2, 16, 102], F32R, tag="p3")
                th3 = pool.tile([128, 2, 14, 100], F32R, tag="h3")
                for kh in range(4):
                    nc.sync.dma_start(out=tc3[:, kh], in_=c_d[1][kh])
                    nc.sync.dma_start(out=twl3[:, kh], in_=wlat_d[1][kh])
                for ih in range(2):
                    nc.sync.dma_start(out=twt3[:, ih], in_=wtr_d[1][ih])
                zero_cols(tm3, 32, [0, 101])
                zero_cols(tp3, 16, [0, 101])

                for ci in range(8):
                    r0 = ci * 4
                    for mo in range(2):
                        ps = pspool.tile([128, 512], F32, tag="ps")
                        for kh in range(4):
                            rhs = tc3[:, kh, r0:r0 + 4, :].rearrange(
                                "p r x -> p (r x)")
                            nc.tensor.matmul(ps[:, :400], twl3[:, kh, mo],
                                             rhs, start=(kh == 0),
                                             stop=(kh == 3))
                        ps5 = ps[:, :400].rearrange(
                            "p (hh r wh s) -> p r hh wh s", r=2, wh=50, s=2)
                        dest5 = tm3[:, mo, r0:r0 + 4, 1:101].rearrange(
                            "p (hh r) (wh s) -> p r hh wh s", r=2, s=2)
                        srow = 1 + r0 // 2
                        srcb = tm4[:, mo, srow:srow + 2, 1:51].unsqueeze(
                            3).broadcast_to([128, 2, 50, 2])
                        for par in range(2):
                            nc.vector.tensor_tensor(
                                dest5[:, par], ps5[:, par], srcb, ALU.add)
                mask_rows(tm3, MOFF["m3"], [0, 1, 30, 31], 1, 101)
                for s in range(2):
                    conv3x3(tp3, [(0, 4), (4, 4), (8, 4), (12, 4)], tm3,
                            14 * s, twt3, tbtr[:, 1], False, 1)
                    mask_rows(tp3, MOFF["p3"] + 14 * s, [0, 15], 1, 101)
                    conv3x3(th3, [(0, 4), (4, 4), (8, 4), (12, 2)], tp3, 0,
                            twr1, tbr1, True, 1)
                    out_head(th3, 14 * 100, 1, s * 1400)

            # ================= L2 =================
            with tc.tile_pool(name="l2", bufs=1) as pool:
                twl2 = pool.tile([128, 2, 2, 128], F32R, tag="wl2")
                twt2 = pool.tile([128, 2, 2, 3, 3, 128], F32R, tag="wt2")
                tm2 = pool.tile([128, 2, 18, 202], F32R, tag="m2")
                tp2 = pool.tile([128, 2, 16, 202], F32R, tag="p2")
                th2 = pool.tile([128, 2, 14, 200], F32R, tag="h2")
                for kh in range(2):
                    nc.sync.dma_start(out=twl2[:, kh], in_=wlat_d[0][kh])
                    nc.sync.dma_start(out=twt2[:, kh], in_=wtr_d[0][kh])
                zero_cols(tm2, 18, [0, 201])
                zero_cols(tp2, 16, [0, 201])
                with tc.tile_pool(name="c2chunk", bufs=4) as c2pool:
                    for s in range(4):
                        for ci in range(9):
                            r0 = ci * 2
                            tcc = c2pool.tile([128, 2, 2, 200], F32R,
                                              tag="c2")
                            for kh in range(2):
                                nc.sync.dma_start(
                                    out=tcc[:, kh],
                                    in_=c_d[0][kh, :,
                                               14 * s + r0:14 * s + r0 + 2,
                                               :])
                            for mo in range(2):
                                ps = pspool.tile([128, 512], F32, tag="ps")
                                for kh in range(2):
                                    rhs = tcc[:, kh].rearrange(
                                        "p r x -> p (r x)")
                                    nc.tensor.matmul(
                                        ps[:, :400], twl2[:, kh, mo], rhs,
                                        start=(kh == 0), stop=(kh == 1))
                                ps5 = ps[:, :400].rearrange(
                                    "p (hh r wh s) -> p r hh wh s",
                                    r=2, wh=100, s=2)
                                dest5 = tm2[:, mo, r0:r0 + 2, 1:201
                                            ].rearrange(
                                    "p (hh r) (wh s) -> p r hh wh s",
                                    r=2, s=2)
                                srow = 1 + (14 * s + r0) // 2
                                srcb = tm3[:, mo, srow:srow + 1, 1:101
                                           ].unsqueeze(3).broadcast_to(
                                    [128, 1, 100, 2])
                                for par in range(2):
                                    nc.vector.tensor_tensor(
                                        dest5[:, par], ps5[:, par], srcb,
                                        ALU.add)
                        mask_rows(tm2, MOFF["m2"] + 14 * s, [0, 1, 16, 17],
                                  1, 201)
                        conv3x3(tp2, [(j, 2) for j in range(0, 16, 2)],
                                tm2, 0, twt2, tbtr[:, 0], False, 0)
                        mask_rows(tp2, MOFF["p2"] + 14 * s, [0, 15], 1, 201)
                        conv3x3(th2, [(j, 2) for j in range(0, 14, 2)],
                                tp2, 0, twr1, tbr1, True, 0)
                        out_head(th2, 14 * 200, 0, s * 2800)

    nc.compile()
    return nc


def _prep_inputs(c2, c3, c4, c5, lat_w, tr_w, rpn_w1, rpn_b1, rpn_w2,
                 rpn_b2, lat_b, tr_b):
    cs = [c2, c3, c4, c5]
    base = {}

    def conv_w(w):
        # [O=256, I=256, 3, 3] -> [ih, 128k, mo, dy, dx, 128m]
        return np.ascontiguousarray(
            w.reshape(2, 128, 2, 128, 3, 3).transpose(2, 3, 0, 4, 5, 1))

    for i in range(4):
        base[f"wlat{i}"] = np.ascontiguousarray(
            lat_w[i].reshape(2, 128, KH[i], 128).transpose(2, 3, 0, 1))
        base[f"wtr{i}"] = conv_w(tr_w[i])
    base["wrpn1"] = conv_w(rpn_w1)
    w2 = np.zeros((2, 128, 16), np.float32)
    w2[:, :, :15] = rpn_w2.reshape(15, 2, 128).transpose(1, 2, 0)
    base["wrpn2"] = w2
    base["btr"] = np.ascontiguousarray(
        np.stack([b.reshape(2, 128).T for b in tr_b]).transpose(0, 1, 2))
    base["brpn1"] = np.ascontiguousarray(rpn_b1.reshape(2, 128).T)
    b2 = np.zeros((128, 16), np.float32)
    b2[:, :15] = rpn_b2[None, :]
    base["brpn2"] = b2
    base["zeros"] = np.zeros((128, 512), np.float32)
    for b in lat_b:
        if np.abs(b).max() != 0:
            raise NotImplementedError("nonzero lateral bias not supported")

    in_maps = []
    starts = [S2, S3, S4, S5]
    m_off = [2, 2, 2, 3]
    for b in range(2):
        for q in range(4):
            m = dict(base)
            for i in range(4):
                r0 = starts[i][q] - m_off[i]
                rows = M_ROWS[i]
                sl = np.zeros((CIN[i], rows, CW[i]), np.float32)
                lo, hi = max(0, r0), min(H[i], r0 + rows)
                if hi > lo:
                    sl[:, lo - r0:hi - r0, :W[i]] = cs[i][b, :, lo:hi, :]
                m[f"c{i}"] = np.ascontiguousarray(
                    sl.reshape(KH[i], 128, rows, CW[i]))
            mk = np.zeros(244, np.float32)
            spans = [(S2[q] - 2, 200, 60), (S2[q] - 1, 200, 58),
                     (S3[q] - 2, 100, 32), (S3[q] - 1, 100, 30),
                     (S4[q] - 2, 50, 18), (S4[q] - 1, 50, 16),
                     (S5[q] - 3, 25, 13), (S5[q] - 2, 25, 11),
                     (S6[q] - 1, 13, 6)]
            off = 0
            for a0, hh, ln in spans:
                for j in range(ln):
                    mk[off + j] = 1.0 if 0 <= a0 + j < hh else 0.0
                off += ln
            m["masks"] = np.broadcast_to(mk, (128, 244)).copy()
            in_maps.append(m)
    return in_maps


def _assemble(results):
    out = np.zeros((2, 159882, 5), np.float32)
    lvl_off = [0, 120000, 150000, 157500, 159375]
    starts = [S2, S3, S4, S5, S6]
    for b in range(2):
        for q in range(4):
            r = results[b * 4 + q]
            for lv in range(5):
                v, w = VALID[lv][q], W[lv]
                o = r[f"o{lv}"].reshape(OWN[lv] * w * 3, 5)
                a0 = lvl_off[lv] + starts[lv][q] * w * 3
                out[b, a0:a0 + v * w * 3] = o[:v * w * 3]
    return out


def kernel(**inputs):
    from concourse.bass_utils import run_bass_kernel_spmd
    if "nc" not in _CACHED:
        _CACHED["nc"] = _build()
    nc = _CACHED["nc"]
    in_maps = _prep_inputs(
        np.asarray(inputs["c2"], np.float32),
        np.asarray(inputs["c3"], np.float32),
        np.asarray(inputs["c4"], np.float32),
        np.asarray(inputs["c5"], np.float32),
        [np.asarray(inputs[f"lat_w{i}"], np.float32) for i in range(4)],
        [np.asarray(inputs[f"tr_w{i}"], np.float32) for i in range(4)],
        np.asarray(inputs["rpn_w1"], np.float32),
        np.asarray(inputs["rpn_b1"], np.float32),
        np.asarray(inputs["rpn_w2"], np.float32),
        np.asarray(inputs["rpn_b2"], np.float32),
        [np.asarray(inputs[f"lat_b{i}"], np.float32) for i in range(4)],
        [np.asarray(inputs[f"tr_b{i}"], np.float32) for i in range(4)],
    )
    res = run_bass_kernel_spmd(nc, in_maps, list(range(N_CORES)))
    return _assemble(res.results)
